# revision 1
# baseline (speedup 1.0000x reference)
"""GCN layer kernel for 8 trn2 NeuronCores (SPMD, single launch).

Math:  out = D^-1/2 (A+I) D^-1/2 X W^T + b
Key identity: the dense layer commutes with the diagonal scalings:
    out = D^-1/2 (A+I) D^-1/2 (X W^T) + b
so we compute U = X@W^T (tiny) first, then one big matmul A @ (d^-1/2 * U).

Distribution: row-shard A across 8 cores (strip = 1024 rows). Each core:
  phase 1: stream its fp32 strip once from HBM; per 128x128 tile,
           PE-transpose (fp32 transpose mode) -> PSUM -> copy to a
           SBUF-resident bf16 A^T strip (16.8MB, fits in 24MB SBUF);
           simultaneously row-sum the natural chunks on VectorE.
  AllGather (the only collective): 1024 local row sums -> full degree.
  phase 2: d^-1/2 via sqrt+reciprocal+Newton; Y = d^-1/2*U (bf16);
           Z = A^T-tiles^T @ Y accumulated in PSUM over 64 k-tiles;
           out = d^-1/2*(Z + Y_local) + b  (self-loop handled exactly).

A is read from HBM exactly once (33.5MB/core ~ 93us at 360GB/s roofline).
"""

import numpy as np
import ml_dtypes

N = 8192          # nodes
F = 128           # in/out feature dim
NCORES = 8
SR = N // NCORES  # strip rows per core = 1024
P = 128           # partitions / tile edge
IT = SR // P      # 8 row tiles per strip
JT = N // P       # 64 contraction tiles
CH = 2048         # chunk columns for DMA
NCH = N // CH     # 4 chunks per row-tile

_CACHE = {}


def _build_nc():
    import concourse.mybir as mybir
    from concourse import bass
    from concourse.tile import TileContext

    f32 = mybir.dt.float32
    bf16 = mybir.dt.bfloat16
    AF = mybir.ActivationFunctionType

    nc = bass.Bass(num_devices=NCORES)

    A_s = nc.declare_dram_parameter("a_strip", [SR, N], f32, False)
    Xt = nc.declare_dram_parameter("xt_bf", [P, N], bf16, False)       # X^T, bf16
    XtL = nc.declare_dram_parameter("xt_loc", [P, SR], bf16, False)    # local cols of X^T
    Wt = nc.declare_dram_parameter("wt", [P, F], f32, False)           # W^T
    Bb = nc.declare_dram_parameter("b_bc", [P, F], f32, False)         # bias bcast to 128 rows
    Idn = nc.declare_dram_parameter("ident", [P, P], f32, False)
    out = nc.declare_dram_parameter("out", [SR, F], f32, True)

    degL = nc.dram_tensor("deg_local", [IT, P], f32)
    degA = nc.dram_tensor("deg_all", [JT, P], f32, addr_space="Shared")

    with TileContext(nc) as tc:
        with tc.tile_pool(name="const", bufs=1) as constp, \
             tc.tile_pool(name="big", bufs=1) as bigp, \
             tc.tile_pool(name="chunks", bufs=2) as chp, \
             tc.tile_pool(name="small", bufs=1) as smallp, \
             tc.tile_pool(name="outs", bufs=3) as outp, \
             tc.tile_pool(name="trps", bufs=3, space="PSUM") as trps, \
             tc.tile_pool(name="zps", bufs=2, space="PSUM") as zps, \
             tc.tile_pool(name="ups", bufs=2, space="PSUM") as ups:

            # ---- constants / small inputs ----
            ident = constp.tile([P, P], f32)
            nc.sync.dma_start(out=ident[:, :], in_=Idn[:, :])
            wt_sb = constp.tile([P, F], f32)
            nc.sync.dma_start(out=wt_sb[:, :], in_=Wt[:, :])
            bb_sb = constp.tile([P, F], f32)
            nc.sync.dma_start(out=bb_sb[:, :], in_=Bb[:, :])
            wt_bf = constp.tile([P, F], bf16)
            nc.vector.tensor_copy(wt_bf[:, :], wt_sb[:, :])

            xt_sb = bigp.tile([P, N], bf16)
            nc.sync.dma_start(out=xt_sb[:, :], in_=Xt[:, :])
            xtl_sb = constp.tile([P, SR], bf16)
            nc.sync.dma_start(out=xtl_sb[:, :], in_=XtL[:, :])

            # ---- persistent big buffers ----
            At = bigp.tile([P, IT * JT * P], bf16)   # transposed strip, bf16
            Yp = bigp.tile([P, N], bf16)             # U then Y' (scaled), per-jt tiles
            Yloc = bigp.tile([P, SR], f32)           # local U then Y'_local (fp32)
            rsp = smallp.tile([P, IT * NCH], f32)    # row-sum partials
            rs = smallp.tile([P, IT], f32)           # local row sums [p, it]

            # ---- U = X @ W^T  (64 small matmuls; overlaps with A streaming) ----
            for jt in range(JT):
                ups_t = ups.tile([P, F], f32)
                nc.tensor.matmul(
                    ups_t[:, :], xt_sb[:, jt * P:(jt + 1) * P], wt_bf[:, :],
                    start=True, stop=True,
                )
                nc.scalar.copy(Yp[:, jt * F:(jt + 1) * F], ups_t[:, :])
            for it in range(IT):
                ups_t = ups.tile([P, F], f32)
                nc.tensor.matmul(
                    ups_t[:, :], xtl_sb[:, it * P:(it + 1) * P], wt_bf[:, :],
                    start=True, stop=True,
                )
                nc.vector.tensor_copy(Yloc[:, it * F:(it + 1) * F], ups_t[:, :])

            # ---- phase 1: stream A strip; transpose + row-sum ----
            ncopy = 0
            for it in range(IT):
                for g in range(NCH):
                    ch = chp.tile([P, CH], f32)
                    nc.sync.dma_start(
                        out=ch[:, :],
                        in_=A_s[it * P:(it + 1) * P, g * CH:(g + 1) * CH],
                    )
                    nc.vector.tensor_reduce(
                        rsp[:, it * NCH + g:it * NCH + g + 1], ch[:, :],
                        axis=mybir.AxisListType.X, op=mybir.AluOpType.add,
                    )
                    for h in range(4):  # 4 psum banks per chunk, 4 tiles each
                        ps = trps.tile([P, 512], f32)
                        for q in range(4):
                            sub = ch[:, (h * 4 + q) * P:(h * 4 + q + 1) * P]
                            nc.tensor.transpose(
                                ps[:, q * P:(q + 1) * P], sub, ident[:, :],
                            )
                        jt0 = g * 16 + h * 4
                        dest = At[:, (it * JT + jt0) * P:(it * JT + jt0 + 4) * P]
                        # ~5/12 of drains on VectorE, rest on ScalarE
                        if ncopy % 12 < 5:
                            nc.vector.tensor_copy(dest, ps[:, :])
                        else:
                            nc.scalar.copy(dest, ps[:, :])
                        ncopy += 1

            # combine row-sum partials -> rs[:, it]
            for it in range(IT):
                nc.vector.tensor_reduce(
                    rs[:, it:it + 1], rsp[:, it * NCH:(it + 1) * NCH],
                    axis=mybir.AxisListType.X, op=mybir.AluOpType.add,
                )

            # ---- AllGather local row sums ----
            ps8 = zps.tile([IT, P], f32, tag="z")
            nc.tensor.transpose(ps8[:, :], rs[:, :], ident[:, :])
            rsT = smallp.tile([IT, P], f32)
            nc.vector.tensor_copy(rsT[:, :], ps8[:, :])
            nc.sync.dma_start(out=degL[:, :], in_=rsT[:, :])
            nc.gpsimd.collective_compute(
                "AllGather", mybir.AluOpType.bypass,
                replica_groups=[list(range(NCORES))],
                ins=[degL[:, :]], outs=[degA[:, :]],
            )
            deg_sb = smallp.tile([JT, P], f32)
            nc.sync.dma_start(out=deg_sb[:, :], in_=degA[:, :])

            # ---- d^-1/2 (global [64,128] and local [128,8]) ----
            def rsqrt_newton(dst, src, pool, shape):
                # dst = (src+1)^-1/2 with one Newton step to fix sqrt LUT error
                sq = pool.tile(shape, f32)
                nc.scalar.activation(sq, src, AF.Sqrt, bias=1.0)
                r0 = pool.tile(shape, f32)
                nc.vector.reciprocal(r0, sq)
                d1 = pool.tile(shape, f32)
                nc.vector.tensor_scalar_add(d1, src, 1.0)
                t = pool.tile(shape, f32)
                nc.vector.tensor_mul(t, r0, r0)
                nc.vector.tensor_mul(t, t, d1)
                nc.scalar.activation(t, t, AF.Copy, bias=1.5, scale=-0.5)
                nc.vector.tensor_mul(dst, r0, t)

            dinvG = smallp.tile([JT, P], f32)
            rsqrt_newton(dinvG[:, :], deg_sb[:, :], smallp, [JT, P])
            dinvL = smallp.tile([P, IT], f32)
            rsqrt_newton(dinvL[:, :], rs[:, :], smallp, [P, IT])

            # transpose dinvG [64,128] -> dinvT [128,64] via padded PE transpose
            dpad = smallp.tile([P, P], f32)
            nc.vector.memset(dpad[:, :], 0.0)
            nc.vector.tensor_copy(dpad[0:JT, :], dinvG[:, :])
            dps = zps.tile([P, P], f32, tag="z")
            nc.tensor.transpose(dps[:, :], dpad[:, :], ident[:, :])
            dinvT = smallp.tile([P, JT], f32)
            nc.vector.tensor_copy(dinvT[:, :], dps[:, 0:JT])

            # ---- scale: Yp <- dinv * U (bf16, in place); Yloc fp32 ----
            for jt in range(JT):
                nc.vector.tensor_scalar_mul(
                    Yp[:, jt * F:(jt + 1) * F], Yp[:, jt * F:(jt + 1) * F],
                    dinvT[:, jt:jt + 1],
                )
            for it in range(IT):
                nc.vector.tensor_scalar_mul(
                    Yloc[:, it * F:(it + 1) * F], Yloc[:, it * F:(it + 1) * F],
                    dinvL[:, it:it + 1],
                )

            # ---- phase 2: Z = A_strip @ Y ; epilogue ----
            for it in range(IT):
                zp = zps.tile([P, F], f32, tag="z")
                for jt in range(JT):
                    nc.tensor.matmul(
                        zp[:, :],
                        At[:, (it * JT + jt) * P:(it * JT + jt + 1) * P],
                        Yp[:, jt * F:(jt + 1) * F],
                        start=(jt == 0), stop=(jt == JT - 1),
                    )
                t1 = outp.tile([P, F], f32)
                nc.vector.tensor_add(t1[:, :], zp[:, :], Yloc[:, it * F:(it + 1) * F])
                nc.vector.tensor_scalar_mul(t1[:, :], t1[:, :], dinvL[:, it:it + 1])
                nc.vector.tensor_add(t1[:, :], t1[:, :], bb_sb[:, :])
                nc.sync.dma_start(out=out[it * P:(it + 1) * P, :], in_=t1[:, :])

    return nc


_NO_SPLIT_TYPES = ("InstEventSemaphore", "InstSemaphore", "InstTrigger")


def _split_drain_waits(nc, max_waits=1):
    """This walrus build only encodes one sem-wait per instruction; hoist
    extras onto preceding same-engine NOPs (monotonic sems => equivalent)."""
    import concourse.mybir as mybir
    for fn in nc.m.functions:
        for blk in fn.blocks:
            newlist = []
            for ins in blk.instructions:
                si = getattr(ins, "sync_info", None)
                tname = type(ins).__name__
                if si is not None and si.on_wait and len(si.on_wait) > max_waits \
                        and not any(tname.startswith(t) for t in _NO_SPLIT_TYPES):
                    waits = list(si.on_wait)
                    for j, w in enumerate(waits[max_waits:]):
                        newlist.append(mybir.InstNoOp(
                            name=f"{ins.name}-w{j}", engine=ins.engine,
                            ins=[], outs=[],
                            sync_info=mybir.SyncInfo(on_wait=[w], on_update=[]),
                        ))
                    si.on_wait = waits[:max_waits]
                newlist.append(ins)
            blk.instructions[:] = newlist


def _get_nc():
    if "nc" not in _CACHE:
        nc = _build_nc()
        _split_drain_waits(nc)
        _CACHE["nc"] = nc
    return _CACHE["nc"]


def _make_in_maps(X, A, W, b):
    bf16 = ml_dtypes.bfloat16
    X = np.ascontiguousarray(np.asarray(X, dtype=np.float32))
    A = np.ascontiguousarray(np.asarray(A, dtype=np.float32))
    W = np.ascontiguousarray(np.asarray(W, dtype=np.float32))
    b = np.ascontiguousarray(np.asarray(b, dtype=np.float32))
    Xt_bf = np.ascontiguousarray(X.T).astype(bf16)
    Wt = np.ascontiguousarray(W.T)
    Bb = np.ascontiguousarray(np.tile(b[None, :], (P, 1)))
    Idn = np.eye(P, dtype=np.float32)
    in_maps = []
    for c in range(NCORES):
        in_maps.append({
            "a_strip": np.ascontiguousarray(A[c * SR:(c + 1) * SR, :]),
            "xt_bf": Xt_bf,
            "xt_loc": np.ascontiguousarray(Xt_bf[:, c * SR:(c + 1) * SR]),
            "wt": Wt,
            "b_bc": Bb,
            "ident": Idn,
        })
    return in_maps


def _install_ntff_hook():
    """This image's antenv lacks axon_hooks; synthesize it so trace=True
    can reach the terminal's NTFF capture via the libaxon ctypes hook."""
    import sys
    import types
    if "antenv.axon_hooks" in sys.modules:
        return
    try:
        from trn_agent_boot.trn_boot import _ntff_profile_via_ctypes
        hook = _ntff_profile_via_ctypes("/opt/axon/libaxon_pjrt.so")
    except Exception:
        hook = None
    mod = types.ModuleType("antenv.axon_hooks")
    mod._hook = hook
    mod.get_axon_ntff_profile_hook = lambda: mod._hook
    def _set(h):
        mod._hook = h
    mod.set_axon_ntff_profile_hook = _set
    sys.modules["antenv.axon_hooks"] = mod
    import antenv
    antenv.axon_hooks = mod
    # the artifact upload needs a bucket this sandbox doesn't have
    import concourse.bass_utils as bu
    bu.upload_artifacts = lambda tmpdir: f"local:{tmpdir}"


def run(X, A, W, b, trace=False, **trace_kwargs):
    """Run on hardware; returns (output, BassKernelResults)."""
    from concourse.bass_utils import run_bass_kernel_spmd
    if trace:
        _install_ntff_hook()
    nc = _get_nc()
    in_maps = _make_in_maps(X, A, W, b)
    res = run_bass_kernel_spmd(nc, in_maps, list(range(NCORES)),
                               trace=trace, **trace_kwargs)
    outs = [np.asarray(res.results[c]["out"], dtype=np.float32)
            for c in range(NCORES)]
    return np.concatenate(outs, axis=0), res


def kernel(X, A, W, b):
    out, _ = run(X, A, W, b, trace=False)
    return out



# revision 2
# speedup vs baseline: 1.6169x; 1.6169x over previous
"""GCN layer kernel for 8 trn2 NeuronCores (SPMD, single launch).

Math:  out = D^-1/2 (A+I) D^-1/2 X W^T + b
Identity: the dense layer commutes with the diagonal scalings:
    out = D^-1/2 (A+I) D^-1/2 (X W^T) + b
so U = X@W^T (tiny) is computed on-chip, then one big matmul A @ (d^-1/2*U).

Sharding: row-shard A (1024 rows/core). Each core receives its strip
PRE-TRANSPOSED to A^T layout and cast to bf16 on the host (16.8MB/core,
half the fp32 bytes; the transposed layout is what the PE contraction
needs, eliminating 512 on-device PE transposes of the old design).

The per-core j-axis (contraction axis) is ROTATED so each core's own
1024 nodes occupy j-tile slots 0..7: slot s holds global tile
(c*8+s)%64.  This makes "local" work uniform across the shared SPMD
program: local degrees (known before the AllGather) scale local Y
tiles and start the big matmul accumulation during the collective.

Per core:
  - stream A^T strip (64 tiles [128,1024] bf16); row sums accumulate on
    PE via a ones-column stationary matmul (2 chains, N=512 halves).
  - U = X@W^T for all 8192 nodes (64 small matmuls, overlaps DMA).
  - AllGather local row sums -> full degree (the only collective).
  - d^-1/2 via sqrt+reciprocal+Newton; a per-core permutation matmul
    maps gathered degrees to rotated slot order.
  - Z^T accumulates in 2 PSUM banks: for each slot s,
    zacc[f, i] += Y_s[j,f].T @ At_s[j,i]   (Y stationary: 64 loads only)
  - epilogue: 8 PE transposes of Z^T -> natural Z, + Y_local,
    * d_i^-1/2, + b, DMA out.
"""

import numpy as np
import ml_dtypes

N = 8192          # nodes
F = 128           # in/out feature dim
NCORES = 8
SR = N // NCORES  # strip rows per core = 1024
P = 128           # partitions / tile edge
IT = SR // P      # 8 row tiles per strip
JT = N // P       # 64 contraction tiles
H = 512           # psum-bank half of a 1024-wide row

_CACHE = {}


def _build_nc():
    import concourse.mybir as mybir
    from concourse import bass
    from concourse.tile import TileContext

    f32 = mybir.dt.float32
    bf16 = mybir.dt.bfloat16
    AF = mybir.ActivationFunctionType

    nc = bass.Bass(num_devices=NCORES)

    A_t = nc.declare_dram_parameter("at", [N, SR], bf16, False)    # rotated A^T strip
    Xt = nc.declare_dram_parameter("xt", [P, N], bf16, False)      # rotated X^T
    Xtl = nc.declare_dram_parameter("xtl", [P, SR], bf16, False)   # local X^T (natural)
    Wt = nc.declare_dram_parameter("wt", [P, F], bf16, False)      # W^T
    Bb = nc.declare_dram_parameter("bb", [P, F], f32, False)       # bias bcast
    Idn = nc.declare_dram_parameter("ident", [P, P], f32, False)
    On1 = nc.declare_dram_parameter("ones1", [P, 1], bf16, False)
    O11 = nc.declare_dram_parameter("one11", [1, 1], f32, False)
    Pm = nc.declare_dram_parameter("perm", [JT, JT], f32, False)   # rot permutation
    out = nc.declare_dram_parameter("out", [SR, F], f32, True)

    degL = nc.dram_tensor("deg_local", [1, SR], f32)
    degA = nc.dram_tensor("deg_all", [JT, P], f32, addr_space="Shared")

    with TileContext(nc) as tc:
        with tc.tile_pool(name="const", bufs=1) as constp, \
             tc.tile_pool(name="ats", bufs=JT) as atp, \
             tc.tile_pool(name="ys", bufs=JT) as yp, \
             tc.tile_pool(name="small", bufs=1) as smallp, \
             tc.tile_pool(name="outs", bufs=3) as outp, \
             tc.tile_pool(name="zacc", bufs=2, space="PSUM") as zps, \
             tc.tile_pool(name="rb", bufs=2, space="PSUM") as rbp, \
             tc.tile_pool(name="ups", bufs=3, space="PSUM") as ups:

            # ---- constants / small inputs ----
            ident = constp.tile([P, P], f32)
            nc.sync.dma_start(out=ident[:, :], in_=Idn[:, :])
            wt_sb = constp.tile([P, F], bf16)
            nc.sync.dma_start(out=wt_sb[:, :], in_=Wt[:, :])
            bb_sb = constp.tile([P, F], f32)
            nc.sync.dma_start(out=bb_sb[:, :], in_=Bb[:, :])
            ones1 = constp.tile([P, 1], bf16)
            nc.sync.dma_start(out=ones1[:, :], in_=On1[:, :])
            one11 = constp.tile([1, 1], f32)
            nc.sync.dma_start(out=one11[:, :], in_=O11[:, :])
            perm_sb = constp.tile([JT, JT], f32)
            nc.sync.dma_start(out=perm_sb[:, :], in_=Pm[:, :])
            xtl_sb = constp.tile([P, SR], bf16)
            nc.sync.dma_start(out=xtl_sb[:, :], in_=Xtl[:, :])
            xt_sb = constp.tile([P, N], bf16)
            nc.sync.dma_start(out=xt_sb[:, :], in_=Xt[:, :])

            # ---- stream the A^T strip ----
            at_t = []
            for s in range(JT):
                t = atp.tile([P, SR], bf16, tag="at")
                nc.sync.dma_start(out=t[:, :], in_=A_t[s * P:(s + 1) * P, :])
                at_t.append(t)

            # ---- U = X @ W^T for all nodes (bf16, unscaled) ----
            y_t = []
            for s in range(JT):
                u_ps = ups.tile([P, F], f32, tag="u")
                nc.tensor.matmul(u_ps[:, :], xt_sb[:, s * P:(s + 1) * P],
                                 wt_sb[:, :], start=True, stop=True)
                yt = yp.tile([P, F], bf16, tag="y")
                if s % 2 == 0:
                    nc.vector.tensor_copy(yt[:, :], u_ps[:, :])
                else:
                    nc.scalar.copy(yt[:, :], u_ps[:, :])
                y_t.append(yt)
            # local U (natural row order), fp32
            yloc = smallp.tile([P, SR], f32)
            for it in range(IT):
                u_ps = ups.tile([P, F], f32, tag="u")
                nc.tensor.matmul(u_ps[:, :], xtl_sb[:, it * P:(it + 1) * P],
                                 wt_sb[:, :], start=True, stop=True)
                nc.vector.tensor_copy(yloc[:, it * F:(it + 1) * F], u_ps[:, :])

            # ---- row sums on PE: rs[0,i] = sum_j At[j,i] ----
            rs0 = rbp.tile([1, H], f32, tag="rb")
            rs1 = rbp.tile([1, H], f32, tag="rb")
            for s in range(JT):
                nc.tensor.matmul(rs0[:, :], ones1[:, :], at_t[s][:, 0:H],
                                 start=(s == 0), stop=(s == JT - 1))
                nc.tensor.matmul(rs1[:, :], ones1[:, :], at_t[s][:, H:SR],
                                 start=(s == 0), stop=(s == JT - 1))
            rs_row = smallp.tile([1, SR], f32)
            nc.vector.tensor_copy(rs_row[:, 0:H], rs0[:, :])
            nc.vector.tensor_copy(rs_row[:, H:SR], rs1[:, :])
            nc.sync.dma_start(out=degL[:, :], in_=rs_row[:, :])

            def rsqrt_newton(dst, src, pool, shape):
                # dst = (src+1)^-1/2 with one Newton step (sqrt LUT refine)
                sq = pool.tile(shape, f32, tag="rn1")
                nc.scalar.activation(sq, src, AF.Sqrt, bias=1.0)
                r0 = pool.tile(shape, f32, tag="rn2")
                nc.vector.reciprocal(r0, sq)
                d1 = pool.tile(shape, f32, tag="rn3")
                nc.vector.tensor_scalar_add(d1, src, 1.0)
                tt = pool.tile(shape, f32, tag="rn4")
                nc.vector.tensor_mul(tt, r0, r0)
                nc.vector.tensor_mul(tt, tt, d1)
                nc.scalar.activation(tt, tt, AF.Copy, bias=1.5, scale=-0.5)
                nc.vector.tensor_mul(dst, r0, tt)

            # ---- local d^-1/2 (pre-AllGather): [1,1024] -> [128,8] via PE ----
            dl_ps = rbp.tile([P, IT], f32, tag="rb")
            for it in range(IT):
                nc.tensor.matmul(dl_ps[:, it:it + 1],
                                 rs_row[0:1, it * P:(it + 1) * P],
                                 one11[:, :], start=True, stop=True)
            rsL = smallp.tile([P, IT], f32)
            nc.vector.tensor_copy(rsL[:, :], dl_ps[:, :])
            dinvL = smallp.tile([P, IT], f32)
            rsqrt_newton(dinvL[:, :], rsL[:, :], smallp, [P, IT])

            # local Y tiles (slots 0..7) scale with local degrees
            for s in range(IT):
                nc.vector.tensor_scalar_mul(
                    y_t[s][:, :], y_t[s][:, :], dinvL[:, s:s + 1])
            for it in range(IT):
                nc.vector.tensor_scalar_mul(
                    yloc[:, it * F:(it + 1) * F], yloc[:, it * F:(it + 1) * F],
                    dinvL[:, it:it + 1])

            # ---- big matmul Z^T = sum_s Y_s^T @ At_s  (locals first) ----
            z0 = zps.tile([P, H], f32, tag="z")
            z1 = zps.tile([P, H], f32, tag="z")
            for s in range(IT):
                nc.tensor.matmul(z0[:, :], y_t[s][:, :], at_t[s][:, 0:H],
                                 start=(s == 0), stop=False)
                nc.tensor.matmul(z1[:, :], y_t[s][:, :], at_t[s][:, H:SR],
                                 start=(s == 0), stop=False)

            # ---- AllGather local row sums -> full degree ----
            nc.gpsimd.collective_compute(
                "AllGather", mybir.AluOpType.bypass,
                replica_groups=[list(range(NCORES))],
                ins=[degL[:, :]], outs=[degA[:, :]],
            )
            deg_sb = smallp.tile([JT, P], f32)
            nc.sync.dma_start(out=deg_sb[:, :], in_=degA[:, :])

            # permute gathered degrees to this core's rotated slot order
            dr_ps = rbp.tile([JT, P], f32, tag="rb")
            nc.tensor.matmul(dr_ps[:, :], perm_sb[:, :], deg_sb[:, :],
                             start=True, stop=True)
            degR = smallp.tile([JT, P], f32)
            nc.vector.tensor_copy(degR[:, :], dr_ps[:, :])
            dinvG = smallp.tile([JT, P], f32)
            rsqrt_newton(dinvG[:, :], degR[:, :], smallp, [JT, P])

            # transpose [64,128] -> [128,64] via padded PE transpose
            dpad = smallp.tile([P, P], f32)
            nc.vector.memset(dpad[:, :], 0.0)
            nc.vector.tensor_copy(dpad[0:JT, :], dinvG[:, :])
            dt_ps = rbp.tile([P, P], f32, tag="rb")
            nc.tensor.transpose(dt_ps[:, :], dpad[:, :], ident[:, :])
            dinvT = smallp.tile([P, JT], f32)
            nc.vector.tensor_copy(dinvT[:, :], dt_ps[:, 0:JT])

            # non-local Y scales + remaining accumulation
            for s in range(IT, JT):
                nc.vector.tensor_scalar_mul(
                    y_t[s][:, :], y_t[s][:, :], dinvT[:, s:s + 1])
                nc.tensor.matmul(z0[:, :], y_t[s][:, :], at_t[s][:, 0:H],
                                 start=False, stop=(s == JT - 1))
                nc.tensor.matmul(z1[:, :], y_t[s][:, :], at_t[s][:, H:SR],
                                 start=False, stop=(s == JT - 1))

            # ---- epilogue: un-transpose Z^T, + Y_local, * d_i^-1/2, + b ----
            ztsb = smallp.tile([P, SR], f32)
            nc.vector.tensor_copy(ztsb[:, 0:H], z0[:, :])
            nc.vector.tensor_copy(ztsb[:, H:SR], z1[:, :])
            for it in range(IT):
                tp = rbp.tile([P, P], f32, tag="rb")
                nc.tensor.transpose(tp[:, :], ztsb[:, it * P:(it + 1) * P],
                                    ident[:, :])
                o = outp.tile([P, F], f32, tag="o")
                nc.vector.tensor_add(o[:, :], tp[:, :],
                                     yloc[:, it * F:(it + 1) * F])
                nc.vector.tensor_scalar_mul(o[:, :], o[:, :], dinvL[:, it:it + 1])
                nc.vector.tensor_add(o[:, :], o[:, :], bb_sb[:, :])
                nc.sync.dma_start(out=out[it * P:(it + 1) * P, :], in_=o[:, :])

    return nc


_NO_SPLIT_TYPES = ("InstEventSemaphore", "InstSemaphore", "InstTrigger")


def _split_drain_waits(nc, max_waits=1):
    """This walrus build only encodes one sem-wait per instruction; hoist
    extras onto preceding same-engine NOPs (monotonic sems => equivalent)."""
    import concourse.mybir as mybir
    for fn in nc.m.functions:
        for blk in fn.blocks:
            newlist = []
            for ins in blk.instructions:
                si = getattr(ins, "sync_info", None)
                tname = type(ins).__name__
                if si is not None and si.on_wait and len(si.on_wait) > max_waits \
                        and not any(tname.startswith(t) for t in _NO_SPLIT_TYPES):
                    waits = list(si.on_wait)
                    for j, w in enumerate(waits[max_waits:]):
                        newlist.append(mybir.InstNoOp(
                            name=f"{ins.name}-w{j}", engine=ins.engine,
                            ins=[], outs=[],
                            sync_info=mybir.SyncInfo(on_wait=[w], on_update=[]),
                        ))
                    si.on_wait = waits[:max_waits]
                newlist.append(ins)
            blk.instructions[:] = newlist


def _get_nc():
    if "nc" not in _CACHE:
        nc = _build_nc()
        _split_drain_waits(nc)
        _CACHE["nc"] = nc
    return _CACHE["nc"]


def _make_in_maps(X, A, W, b):
    bf16 = ml_dtypes.bfloat16
    X = np.ascontiguousarray(np.asarray(X, dtype=np.float32))
    A = np.ascontiguousarray(np.asarray(A, dtype=np.float32))
    W = np.ascontiguousarray(np.asarray(W, dtype=np.float32))
    b = np.ascontiguousarray(np.asarray(b, dtype=np.float32))
    At_bf = np.asarray(A.T, dtype=bf16)          # [N, N] bf16, column c-strips
    Xt_bf = np.ascontiguousarray(X.T).astype(bf16)
    Wt_bf = np.ascontiguousarray(W.T).astype(bf16)
    Bb = np.ascontiguousarray(np.tile(b[None, :], (P, 1)))
    Idn = np.eye(P, dtype=np.float32)
    On1 = np.ones((P, 1), dtype=bf16)
    O11 = np.ones((1, 1), dtype=np.float32)
    in_maps = []
    for c in range(NCORES):
        at_strip = At_bf[:, c * SR:(c + 1) * SR]           # [8192, 1024]
        at_rot = np.ascontiguousarray(np.roll(at_strip, -c * SR, axis=0))
        xt_rot = np.ascontiguousarray(np.roll(Xt_bf, -c * SR, axis=1))
        pm = np.zeros((JT, JT), dtype=np.float32)
        for s in range(JT):
            pm[(c * IT + s) % JT, s] = 1.0
        in_maps.append({
            "at": at_rot,
            "xt": xt_rot,
            "xtl": np.ascontiguousarray(Xt_bf[:, c * SR:(c + 1) * SR]),
            "wt": Wt_bf,
            "bb": Bb,
            "ident": Idn,
            "ones1": On1,
            "one11": O11,
            "perm": pm,
        })
    return in_maps


def _install_ntff_hook():
    """This image's antenv lacks axon_hooks; synthesize it so trace=True
    can reach the terminal's NTFF capture via the libaxon ctypes hook."""
    import sys
    import types
    if "antenv.axon_hooks" in sys.modules:
        return
    try:
        from trn_agent_boot.trn_boot import _ntff_profile_via_ctypes
        hook = _ntff_profile_via_ctypes("/opt/axon/libaxon_pjrt.so")
    except Exception:
        hook = None
    mod = types.ModuleType("antenv.axon_hooks")
    mod._hook = hook
    mod.get_axon_ntff_profile_hook = lambda: mod._hook

    def _set(h):
        mod._hook = h
    mod.set_axon_ntff_profile_hook = _set
    sys.modules["antenv.axon_hooks"] = mod
    import antenv
    antenv.axon_hooks = mod
    # the artifact upload needs a bucket this sandbox doesn't have
    import concourse.bass_utils as bu
    bu.upload_artifacts = lambda tmpdir: f"local:{tmpdir}"


def run(X, A, W, b, trace=False, **trace_kwargs):
    """Run on hardware; returns (output, BassKernelResults)."""
    from concourse.bass_utils import run_bass_kernel_spmd
    if trace:
        _install_ntff_hook()
    nc = _get_nc()
    in_maps = _make_in_maps(X, A, W, b)
    res = run_bass_kernel_spmd(nc, in_maps, list(range(NCORES)),
                               trace=trace, **trace_kwargs)
    outs = [np.asarray(res.results[c]["out"], dtype=np.float32)
            for c in range(NCORES)]
    return np.concatenate(outs, axis=0), res


def kernel(X, A, W, b):
    out, _ = run(X, A, W, b, trace=False)
    return out
